# revision 12
# baseline (speedup 1.0000x reference)
"""MultiHeadAttention Trainium2 Bass kernel, 8-core SPMD. v2.

Problem: B=4, S=2048, E=2048, H=16, Dh=128; reshape-based (not transposed)
head split:  q = (x@Wq).reshape(B,H,S,Dh) etc., softmax over the QUERY axis,
out = attn.reshape(B,S,E).

Key structure (same as v1): flattening (B,S) rows, row-block gp (128 rows)
of x@W is exactly head pair gp=(b,h): Qh = Y[128gp:128gp+128,:].reshape(2048,128).
Each of the 8 cores handles 8 consecutive pairs -> core c gets contiguous
x rows [1024c:1024c+1024) and produces the same output rows. No collectives.

Per-core internal q/k index permutation (order-free since softmax reduces
over q): f = j*128 + s  <->  q = 16s + j.

v2 changes vs v1 baseline:
- ONE group of 8 pairs: Wq/Wk/Wv each streamed once (was twice).
- Wv + V-projection in bf16 (V path is linear; softmax argmax not affected).
- 1/sqrt(128) scale folded into Wq on the host.
- Q^T/K^T spilled to DRAM f32r, reloaded per pair (d-major layout).
- PSUM: psSC 3x[128,1024] + psAT 1x[128,1024] = 8 banks; scores pipeline
  depth 3; attention accumulated per 8-kj group into psAT, drained to SBUF.
- Attention processed two pairs at a time (staggered) so the serial
  scores->max->min->exp chain of one pair overlaps the other's work.
- Softmax stats: one column-max on GpSimd pool_max, one on DVE; combined
  with scalar_tensor_tensor; exp on ACT with accumulated row sums.
- Copies balanced across DVE/ACT; DMAs split across sync/gpsimd queues.
"""

import numpy as np
import ml_dtypes
from contextlib import ExitStack

import concourse.bass as bass
import concourse.tile as tile
from concourse import bacc, mybir
from concourse.bass import ds, ts
from concourse.bass_utils import run_bass_kernel_spmd
from concourse.masks import make_identity

F32 = mybir.dt.float32
F32R = mybir.dt.float32r
BF16 = mybir.dt.bfloat16
AX = mybir.AxisListType.X
EXP = mybir.ActivationFunctionType.Exp
MULT = mybir.AluOpType.mult
MIN = mybir.AluOpType.min

P = 128
NPAIR = 8          # (b,h) pairs per core
NJ = 16            # 128-blocks in E / contraction
SCALE = 1.0 / np.sqrt(128.0)
POOL_MAX = True    # one of the two column maxes on the GpSimd pool engine

_cache = {}


class _PairState:
    __slots__ = ("gp", "qt", "kt", "yv", "acc", "softs", "vss", "lsums")

    def __init__(self, gp, qt, kt, yv, acc):
        self.gp, self.qt, self.kt, self.yv, self.acc = gp, qt, kt, yv, acc
        self.softs, self.vss, self.lsums = {}, {}, {}


def _emit(nc, tc, ctx, xl, wq, wk, wv, idr, out):
    sb = ctx.enter_context
    dram = sb(tc.tile_pool(name="dram", bufs=1, space="DRAM"))
    # d-major spill layout: [d, pair, j, s]
    qsp = dram.tile([P, NPAIR, NJ, P], F32R, tag="qsp")
    ksp = dram.tile([P, NPAIR, NJ, P], F32R, tag="ksp")

    # PSUM: 3x[128,1024] + 1x[128,1024] = 8 banks
    psSC = sb(tc.tile_pool(name="pssc", bufs=3, space="PSUM"))
    psAT = sb(tc.tile_pool(name="psat", bufs=1, space="PSUM"))

    pCONST = sb(tc.tile_pool(name="pconst", bufs=1))
    ident = pCONST.tile([P, P], F32, tag="ident")
    make_identity(nc, ident[:])
    identr = pCONST.tile([P, P], F32R, tag="identr")
    nc.sync.dma_start(identr[:], idr)

    pYV = sb(tc.tile_pool(name="pyv", bufs=NPAIR))
    yv_tiles = [
        pYV.tile([P, NJ * P], BF16, tag="yv", name=f"yv{i}") for i in range(NPAIR)
    ]

    with tc.tile_pool(name="pxtg", bufs=1) as pXTG:
        xtg = pXTG.tile([P, NJ, NPAIR, P], F32R, tag="xtg")
        with tc.tile_pool(name="pxin", bufs=2) as pIN, \
             tc.tile_pool(name="pxtg16", bufs=1) as pXTG16:
            xtg16 = pXTG16.tile([P, NJ, NPAIR, P], BF16, tag="xtg16")
            # ---- phase X: transpose x row-blocks into [d, kb, pair, s] ----
            for pi in range(NPAIR):
                xt = pIN.tile([P, NJ * P], F32R, tag="xt")
                nc.sync.dma_start(xt[:], xl[ds(pi * P, P), :])
                for jj in range(2):
                    pt = psSC.tile([P, 1024], F32, tag="sc")
                    for i in range(8):
                        j = jj * 8 + i
                        nc.tensor.transpose(
                            pt[:, ds(i * P, P)].bitcast(F32R),
                            xt[:, ds(j * P, P)], identr[:],
                        )
                    nc.vector.tensor_copy(
                        xtg[:, ts(jj, 8), pi, :],
                        pt[:].bitcast(F32R).rearrange("p (a b) -> p a b", a=8),
                    )
                    nc.scalar.copy(
                        xtg16[:, ts(jj, 8), pi, :],
                        pt[:].rearrange("p (a b) -> p a b", a=8),
                    )
            # ---- phase V: Yv = X @ Wv in bf16, natural [s, e] layout ----
            with tc.tile_pool(name="pwv", bufs=2) as pWV:
                for c in range(4):
                    wvt = pWV.tile([P, NJ, 512], BF16, tag="wv")
                    nc.sync.dma_start(wvt[:], wv[c])
                    for pi in range(NPAIR):
                        ps = psSC.tile([P, 1024], F32, tag="sc")
                        for kb in range(NJ):
                            nc.tensor.matmul(
                                ps[:, ds(0, 512)], xtg16[:, kb, pi], wvt[:, kb],
                                start=(kb == 0), stop=(kb == NJ - 1),
                            )
                        nc.scalar.copy(yv_tiles[pi][:, ds(c * 512, 512)], ps[:, ds(0, 512)])

        # ---- phase QK: Q^T/K^T projections, spill to DRAM ----
        with tc.tile_pool(name="pw", bufs=3) as pW, \
             tc.tile_pool(name="pstg", bufs=4) as pSTG:
            ci = 0
            for wd, sp in ((wq, qsp), (wk, ksp)):
                for j in range(NJ):
                    wt = pW.tile([P, NJ, P], F32R, tag="wqk")
                    nc.sync.dma_start(wt[:], wd[j])
                    ps = psSC.tile([P, 1024], F32, tag="sc")
                    for h in range(2):
                        for kb in range(NJ):
                            nc.tensor.matmul(
                                ps[:, ds(h * 512, 512)], wt[:, kb],
                                xtg[:, kb, ds(h * 4, 4), :],
                                start=(kb == 0), stop=(kb == NJ - 1),
                            )
                    stg = pSTG.tile([P, NPAIR, P], F32R, tag="stg")
                    if ci % 2 == 0:
                        nc.vector.tensor_copy(
                            stg[:], ps[:].rearrange("p (g s) -> p g s", g=NPAIR)
                        )
                    else:
                        nc.scalar.copy(
                            stg[:], ps[:].rearrange("p (g s) -> p g s", g=NPAIR)
                        )
                    ci += 1
                    nc.gpsimd.dma_start(sp[:, :, j, :], stg[:])

    # ---- attention, two pairs staggered ----
    with tc.tile_pool(name="pqt", bufs=4) as pQT, \
         tc.tile_pool(name="pkt", bufs=4) as pKT, \
         tc.tile_pool(name="psoft", bufs=18) as pSOFT, \
         tc.tile_pool(name="pvs", bufs=18) as pVS, \
         tc.tile_pool(name="pacc", bufs=2) as pACC, \
         tc.tile_pool(name="pst", bufs=36) as pST:

        def load(gp):
            qt = pQT.tile([P, NJ, P], F32R, tag="qt", name=f"qt{gp}")
            nc.sync.dma_start(qt[:], qsp[:, gp])
            kt = pKT.tile([P, NJ, P], F32R, tag="kt", name=f"kt{gp}")
            nc.sync.dma_start(kt[:], ksp[:, gp])
            acc = pACC.tile([P, NJ * P], F32, tag="acc", name=f"acc{gp}")
            return _PairState(gp, qt, kt, yv_tiles[gp], acc)

        def step_pre(st, kj):
            """Scores matmuls, column maxes, combine, exps (with accum sums)."""
            soft = pSOFT.tile([P, 2048], BF16, tag="soft")
            pss = []
            for h in range(2):
                ps = psSC.tile([P, 1024], F32, tag="sc")
                for c in range(2):
                    nc.tensor.matmul(
                        ps[:, ds(c * 512, 512)], st.kt[:, kj],
                        st.qt[:, ts(h * 2 + c, 4)], start=True, stop=True,
                    )
                pss.append(ps)
            ng = pST.tile([P, 1], F32, tag="ng")
            nm0 = pST.tile([P, 1], F32, tag="m0")
            nc.vector.reduce_max(nm0[:], pss[0][:], axis=AX, negate=True)
            nm1 = pST.tile([P, 1], F32, tag="nm1")
            nc.vector.reduce_max(nm1[:], pss[1][:], axis=AX, negate=True)
            nc.vector.tensor_tensor(ng[:], nm0[:], nm1[:], MIN)
            lsum = pST.tile([P, 2], F32, tag="ls")
            for h in range(2):
                nc.scalar.activation(
                    soft[:, ds(h * 1024, 1024)], pss[h][:], EXP,
                    bias=ng[:], scale=1.0, accum_out=lsum[:, ds(h, 1)],
                )
            st.softs[kj] = soft
            st.lsums[kj] = lsum

        def step_post(st, kj):
            """Deferred sum/reciprocal/V-scale: issued one step later so these
            ACT-dependent ops don't block the next step's maxes in the
            in-order DVE queue."""
            lsum = st.lsums.pop(kj)
            lt = pST.tile([P, 1], F32, tag="lt")
            nc.vector.reduce_sum(lt[:], lsum[:], axis=AX)
            rcp = pST.tile([P, 1], F32, tag="rcp")
            nc.vector.reciprocal(rcp[:], lt[:])
            vs = pVS.tile([P, P], BF16, tag="vs")
            nc.vector.tensor_scalar_mul(vs[:], st.yv[:, ts(kj, P)], rcp[:])
            st.vss[kj] = vs

        def burst(st, g0):
            for h in range(2):
                pa = psAT.tile([P, 1024], F32, tag="at")
                for i in range(8):
                    kj = g0 + i
                    for c in range(2):
                        nc.tensor.matmul(
                            pa[:, ds(c * 512, 512)], st.vss[kj][:],
                            st.softs[kj][:, ds(h * 1024 + c * 512, 512)],
                            start=(i == 0), stop=(i == 7),
                        )
                if g0 == 0:
                    nc.scalar.copy(st.acc[:, ds(h * 1024, 1024)], pa[:])
                else:
                    nc.vector.tensor_add(
                        st.acc[:, ds(h * 1024, 1024)],
                        st.acc[:, ds(h * 1024, 1024)], pa[:],
                    )
            if g0 == 8:
                st.softs.clear()
                st.vss.clear()

        def finish(st):
            acc = st.acc
            for jj in range(2):
                pt = psAT.tile([P, 1024], F32, tag="at")
                for i in range(8):
                    cblk = jj * 8 + i
                    nc.tensor.transpose(
                        pt[:, ds(i * P, P)], acc[:, ds(cblk * P, P)], ident[:]
                    )
                nc.vector.tensor_copy(acc[:, ds(jj * 1024, 1024)], pt[:])
            nc.sync.dma_start(out[ds(st.gp * P, P), :], acc[:])

        states = {}
        states[0], states[1] = load(0), load(1)
        for d in range(4):
            pa_, pb_ = 2 * d, 2 * d + 1
            if d < 3:
                states[pa_ + 2], states[pb_ + 2] = load(pa_ + 2), load(pb_ + 2)
            stA, stB = states.pop(pa_), states.pop(pb_)
            for kj in range(NJ):
                step_pre(stA, kj)
                if kj > 0:
                    step_post(stA, kj - 1)
                if kj == 9:
                    burst(stA, 0)
                step_pre(stB, kj)
                if kj > 0:
                    step_post(stB, kj - 1)
                if kj == 9:
                    burst(stB, 0)
            step_post(stA, NJ - 1)
            burst(stA, 8)
            finish(stA)
            step_post(stB, NJ - 1)
            burst(stB, 8)
            finish(stB)


def build(compile=True):
    key = ("nc_v2", compile)
    if key in _cache:
        return _cache[key]
    nc = bacc.Bacc("TRN2", target_bir_lowering=False, debug=False)
    xl = nc.dram_tensor("xl", [NPAIR * P, 2048], F32R, kind="ExternalInput").ap()
    wq = nc.dram_tensor("wq", [NJ, P, NJ, P], F32R, kind="ExternalInput").ap()
    wk = nc.dram_tensor("wk", [NJ, P, NJ, P], F32R, kind="ExternalInput").ap()
    wv = nc.dram_tensor("wv", [4, P, NJ, 512], BF16, kind="ExternalInput").ap()
    idr = nc.dram_tensor("idr", [P, P], F32R, kind="ExternalInput").ap()
    out = nc.dram_tensor("out", [NPAIR * P, 2048], F32, kind="ExternalOutput").ap()
    with tile.TileContext(nc) as tc:
        with ExitStack() as ctx:
            _emit(nc, tc, ctx, xl, wq, wk, wv, idr, out)
    if compile:
        nc.compile()
    _cache[key] = nc
    return nc


def kernel(x, w_query, w_key, w_value, _want_trace=False):
    x = np.ascontiguousarray(np.asarray(x, np.float32))
    wqa = np.ascontiguousarray(np.asarray(w_query, np.float32))
    wka = np.ascontiguousarray(np.asarray(w_key, np.float32))
    wva = np.ascontiguousarray(np.asarray(w_value, np.float32))
    B, S, E = x.shape
    xf = x.reshape(B * S, E)
    nc = build()
    rows = NPAIR * P
    wq_t = np.ascontiguousarray(
        (wqa * SCALE).reshape(NJ, P, NJ, P).transpose(2, 1, 0, 3)
    )
    wk_t = np.ascontiguousarray(wka.reshape(NJ, P, NJ, P).transpose(2, 1, 0, 3))
    wv_t = np.ascontiguousarray(
        wva.reshape(NJ, P, 4, 512).transpose(2, 1, 0, 3).astype(ml_dtypes.bfloat16)
    )
    eye = np.eye(P, dtype=np.float32)
    in_maps = [
        dict(xl=np.ascontiguousarray(xf[c * rows:(c + 1) * rows]),
             wq=wq_t, wk=wk_t, wv=wv_t, idr=eye)
        for c in range(8)
    ]
    res = run_bass_kernel_spmd(nc, in_maps, core_ids=list(range(8)),
                               trace=_want_trace)
    outf = np.concatenate([r["out"] for r in res.results], axis=0)
    if _want_trace:
        kernel.last_result = res
    return outf.reshape(B, S, E)


# revision 14
# speedup vs baseline: 1.1773x; 1.1773x over previous
"""MultiHeadAttention Trainium2 Bass kernel, 8-core SPMD. v3.

Problem: B=4, S=2048, E=2048, H=16, Dh=128; reshape-based (not transposed)
head split:  q = (x@Wq).reshape(B,H,S,Dh) etc., softmax over the QUERY axis,
out = attn.reshape(B,S,E).

Key structure: flattening (B,S) rows, row-block gp (128 rows) of x@W is
exactly head pair gp=(b,h): Qh = Y[128gp:128gp+128,:].reshape(2048,128).
Each of the 8 cores handles 8 consecutive pairs -> core c gets contiguous
x rows [1024c:1024c+1024) and produces the same output rows. No collectives.

Per-core internal q/k index permutation (order-free since softmax reduces
over q): f = j*128 + s  <->  q = 16s + j.

v3 design:
- ONE group of 8 pairs: Wq/Wk/Wv each streamed once.
- X-transpose and V-projection interleaved per pair (Wv bf16, resident).
- 1/sqrt(128) scale folded into Wq on the host.
- Q^T/K^T spilled to DRAM f32r (d-major), reloaded per pair.
- PSUM: psSC 3x[128,1024] (scores, 1.5-step lookahead) + psAT 2x[128,512]
  (attention accumulated in quarter-tiles, drains overlap next quarter's
  matmuls) = 8 banks.
- Attention two pairs staggered; sum/reciprocal/V-scale deferred one step
  (step_post) so ACT-dependent ops don't block next step's maxes in the
  in-order DVE queue; burst quarters spread over steps 9..12, drains
  deferred and alternated DVE/ACT.
- DMA queues: weights/x/qt/kt/out on sync, spills on gpsimd.
"""

import numpy as np
import ml_dtypes
from contextlib import ExitStack

import concourse.bass as bass
import concourse.tile as tile
from concourse import bacc, mybir
from concourse.bass import ds, ts
from concourse.bass_utils import run_bass_kernel_spmd
from concourse.masks import make_identity

F32 = mybir.dt.float32
F32R = mybir.dt.float32r
BF16 = mybir.dt.bfloat16
AX = mybir.AxisListType.X
EXP = mybir.ActivationFunctionType.Exp
MIN = mybir.AluOpType.min

P = 128
NPAIR = 8          # (b,h) pairs per core
NJ = 16            # 128-blocks in E / contraction
SCALE = 1.0 / np.sqrt(128.0)

_cache = {}


class _PairState:
    __slots__ = ("gp", "qt", "kt", "yv", "acc", "softs", "vss", "lsums", "pas")

    def __init__(self, gp, qt, kt, yv, acc):
        self.gp, self.qt, self.kt, self.yv, self.acc = gp, qt, kt, yv, acc
        self.softs, self.vss, self.lsums = {}, {}, {}
        self.pas = {}


def _emit(nc, tc, ctx, xl, wq, wk, wv, idr, out):
    sb = ctx.enter_context
    dram = sb(tc.tile_pool(name="dram", bufs=1, space="DRAM"))
    # d-major spill layout: [d, pair, j, s]
    qsp = dram.tile([P, NPAIR, NJ, P], F32R, tag="qsp")
    ksp = dram.tile([P, NPAIR, NJ, P], F32R, tag="ksp")

    # PSUM: 3x[128,1024] + 2x[128,512] = 8 banks
    psSC = sb(tc.tile_pool(name="pssc", bufs=3, space="PSUM"))
    psAT = sb(tc.tile_pool(name="psat", bufs=2, space="PSUM"))

    pCONST = sb(tc.tile_pool(name="pconst", bufs=1))
    ident = pCONST.tile([P, P], F32, tag="ident")
    make_identity(nc, ident[:])
    identr = pCONST.tile([P, P], F32R, tag="identr")
    nc.sync.dma_start(identr[:], idr)

    pYV = sb(tc.tile_pool(name="pyv", bufs=NPAIR))
    yv_tiles = [
        pYV.tile([P, NJ * P], BF16, tag="yv", name=f"yv{i}") for i in range(NPAIR)
    ]

    with tc.tile_pool(name="pxtg", bufs=1) as pXTG:
        xtg = pXTG.tile([P, NJ, NPAIR, P], F32R, tag="xtg")
        # ---- phase XV: per pair, transpose x block then project V ----
        with tc.tile_pool(name="pxin", bufs=2) as pIN, \
             tc.tile_pool(name="pxt16", bufs=2) as pXT16, \
             tc.tile_pool(name="pwv", bufs=4) as pWV:
            wvts = []
            for c in range(4):
                wvt = pWV.tile([P, NJ, 512], BF16, tag="wv", name=f"wv{c}")
                nc.sync.dma_start(wvt[:], wv[c])
                wvts.append(wvt)
            for pi in range(NPAIR):
                xt = pIN.tile([P, NJ * P], F32R, tag="xt")
                nc.sync.dma_start(xt[:], xl[ds(pi * P, P), :])
                # xt16: [in-dim-block kb, s] transposed copy of this pair, bf16
                xt16 = pXT16.tile([P, NJ, P], BF16, tag="xt16")
                for jj in range(2):
                    pt = psSC.tile([P, 1024], F32, tag="sc")
                    for i in range(8):
                        j = jj * 8 + i
                        nc.tensor.transpose(
                            pt[:, ds(i * P, P)].bitcast(F32R),
                            xt[:, ds(j * P, P)], identr[:],
                        )
                    nc.vector.tensor_copy(
                        xtg[:, ts(jj, 8), pi, :],
                        pt[:].bitcast(F32R).rearrange("p (a b) -> p a b", a=8),
                    )
                    nc.scalar.copy(
                        xt16[:, ts(jj, 8), :],
                        pt[:].rearrange("p (a b) -> p a b", a=8),
                    )
                for c in range(4):
                    ps = psSC.tile([P, 1024], F32, tag="sc")
                    for kb in range(NJ):
                        nc.tensor.matmul(
                            ps[:, ds(0, 512)], xt16[:, kb], wvts[c][:, kb],
                            start=(kb == 0), stop=(kb == NJ - 1),
                        )
                    nc.scalar.copy(yv_tiles[pi][:, ds(c * 512, 512)], ps[:, ds(0, 512)])

        # ---- phase QK: Q^T/K^T projections, spill to DRAM ----
        with tc.tile_pool(name="pw", bufs=3) as pW, \
             tc.tile_pool(name="pstg", bufs=4) as pSTG:
            ci = 0
            for wd, sp in ((wq, qsp), (wk, ksp)):
                for j in range(NJ):
                    wt = pW.tile([P, NJ, P], F32R, tag="wqk")
                    nc.sync.dma_start(wt[:], wd[j])
                    ps = psSC.tile([P, 1024], F32, tag="sc")
                    for h in range(2):
                        for kb in range(NJ):
                            nc.tensor.matmul(
                                ps[:, ds(h * 512, 512)], wt[:, kb],
                                xtg[:, kb, ds(h * 4, 4), :],
                                start=(kb == 0), stop=(kb == NJ - 1),
                            )
                    stg = pSTG.tile([P, NPAIR, P], F32R, tag="stg")
                    if ci % 2 == 0:
                        nc.vector.tensor_copy(
                            stg[:], ps[:].rearrange("p (g s) -> p g s", g=NPAIR)
                        )
                    else:
                        nc.scalar.copy(
                            stg[:], ps[:].rearrange("p (g s) -> p g s", g=NPAIR)
                        )
                    ci += 1
                    nc.gpsimd.dma_start(sp[:, :, j, :], stg[:])

    # ---- attention, two pairs staggered ----
    with tc.tile_pool(name="pqt", bufs=4) as pQT, \
         tc.tile_pool(name="pkt", bufs=4) as pKT, \
         tc.tile_pool(name="psoft", bufs=18) as pSOFT, \
         tc.tile_pool(name="pvs", bufs=18) as pVS, \
         tc.tile_pool(name="pacc", bufs=2) as pACC, \
         tc.tile_pool(name="pst", bufs=36) as pST:

        def load(gp):
            qt = pQT.tile([P, NJ, P], F32R, tag="qt", name=f"qt{gp}")
            nc.sync.dma_start(qt[:], qsp[:, gp])
            kt = pKT.tile([P, NJ, P], F32R, tag="kt", name=f"kt{gp}")
            nc.sync.dma_start(kt[:], ksp[:, gp])
            acc = pACC.tile([P, NJ * P], F32, tag="acc", name=f"acc{gp}")
            return _PairState(gp, qt, kt, yv_tiles[gp], acc)

        def step_pre(st, kj):
            """Scores matmuls, column maxes, combine, exps (+accum sums)."""
            soft = pSOFT.tile([P, 2048], BF16, tag="soft")
            pss = []
            for h in range(2):
                ps = psSC.tile([P, 1024], F32, tag="sc")
                for c in range(2):
                    nc.tensor.matmul(
                        ps[:, ds(c * 512, 512)], st.kt[:, kj],
                        st.qt[:, ts(h * 2 + c, 4)], start=True, stop=True,
                    )
                pss.append(ps)
            ng = pST.tile([P, 1], F32, tag="ng")
            nm0 = pST.tile([P, 1], F32, tag="m0")
            nc.vector.reduce_max(nm0[:], pss[0][:], axis=AX, negate=True)
            nm1 = pST.tile([P, 1], F32, tag="nm1")
            nc.vector.reduce_max(nm1[:], pss[1][:], axis=AX, negate=True)
            nc.vector.tensor_tensor(ng[:], nm0[:], nm1[:], MIN)
            lsum = pST.tile([P, 2], F32, tag="ls")
            for h in range(2):
                nc.scalar.activation(
                    soft[:, ds(h * 1024, 1024)], pss[h][:], EXP,
                    bias=ng[:], scale=1.0, accum_out=lsum[:, ds(h, 1)],
                )
            st.softs[kj] = soft
            st.lsums[kj] = lsum

        def step_post(st, kj):
            """Deferred sum/reciprocal/V-scale (one step later so these
            ACT-dependent ops don't stall the in-order DVE queue)."""
            lsum = st.lsums.pop(kj)
            lt = pST.tile([P, 1], F32, tag="lt")
            nc.vector.reduce_sum(lt[:], lsum[:], axis=AX)
            rcp = pST.tile([P, 1], F32, tag="rcp")
            nc.vector.reciprocal(rcp[:], lt[:])
            vs = pVS.tile([P, P], BF16, tag="vs")
            nc.vector.tensor_scalar_mul(vs[:], st.yv[:, ts(kj, P)], rcp[:])
            st.vss[kj] = vs

        qno = [0]

        def quarter(st, g0, h, c):
            """One [128,512] attention-accumulation quarter over 8 kj."""
            pa = psAT.tile([P, 512], F32, tag="at")
            for i in range(8):
                kj = g0 + i
                nc.tensor.matmul(
                    pa[:], st.vss[kj][:],
                    st.softs[kj][:, ds(h * 1024 + c * 512, 512)],
                    start=(i == 0), stop=(i == 7),
                )
            st.pas[(g0, h, c)] = pa

        def drain(st, g0, h, c):
            pa = st.pas.pop((g0, h, c))
            dst = st.acc[:, ds(h * 1024 + c * 512, 512)]
            if g0 == 0:
                if qno[0] % 2 == 0:
                    nc.scalar.copy(dst, pa[:])
                else:
                    nc.vector.tensor_copy(dst, pa[:])
            else:
                nc.vector.tensor_add(dst, dst, pa[:])
            qno[0] += 1

        QSCHED = {9: (0, 0), 10: (0, 1), 11: (1, 0), 12: (1, 1)}

        def finish(st):
            acc = st.acc
            for jj in range(4):
                pt = psAT.tile([P, 512], F32, tag="at")
                for i in range(4):
                    cblk = jj * 4 + i
                    nc.tensor.transpose(
                        pt[:, ds(i * P, P)], acc[:, ds(cblk * P, P)], ident[:]
                    )
                if jj % 2 == 0:
                    nc.scalar.copy(acc[:, ds(jj * 512, 512)], pt[:])
                else:
                    nc.vector.tensor_copy(acc[:, ds(jj * 512, 512)], pt[:])
            nc.sync.dma_start(out[ds(st.gp * P, P), :], acc[:])

        states = {}
        states[0], states[1] = load(0), load(1)
        for d in range(4):
            pa_, pb_ = 2 * d, 2 * d + 1
            if d < 3:
                states[pa_ + 2], states[pb_ + 2] = load(pa_ + 2), load(pb_ + 2)
            stA, stB = states.pop(pa_), states.pop(pb_)
            for kj in range(NJ):
                step_pre(stA, kj)
                if kj > 0:
                    step_post(stA, kj - 1)
                if kj in QSCHED:
                    h, c = QSCHED[kj]
                    quarter(stA, 0, h, c)
                if kj - 1 in QSCHED:
                    drain(stA, 0, *QSCHED[kj - 1])
                step_pre(stB, kj)
                if kj > 0:
                    step_post(stB, kj - 1)
                if kj in QSCHED:
                    h, c = QSCHED[kj]
                    quarter(stB, 0, h, c)
                if kj - 1 in QSCHED:
                    drain(stB, 0, *QSCHED[kj - 1])
            step_post(stA, NJ - 1)
            step_post(stB, NJ - 1)
            # tail: group-1 quarters, interleaved A/B, drains one behind
            tail = [(0, 0), (0, 1), (1, 0), (1, 1)]
            for qi, (h, c) in enumerate(tail):
                quarter(stA, 8, h, c)
                if qi > 0:
                    drain(stA, 8, *tail[qi - 1])
                quarter(stB, 8, h, c)
                if qi > 0:
                    drain(stB, 8, *tail[qi - 1])
            drain(stA, 8, 1, 1)
            finish(stA)
            drain(stB, 8, 1, 1)
            finish(stB)


def build(compile=True):
    key = ("nc_v3", compile)
    if key in _cache:
        return _cache[key]
    nc = bacc.Bacc("TRN2", target_bir_lowering=False, debug=False)
    xl = nc.dram_tensor("xl", [NPAIR * P, 2048], F32R, kind="ExternalInput").ap()
    wq = nc.dram_tensor("wq", [NJ, P, NJ, P], F32R, kind="ExternalInput").ap()
    wk = nc.dram_tensor("wk", [NJ, P, NJ, P], F32R, kind="ExternalInput").ap()
    wv = nc.dram_tensor("wv", [4, P, NJ, 512], BF16, kind="ExternalInput").ap()
    idr = nc.dram_tensor("idr", [P, P], F32R, kind="ExternalInput").ap()
    out = nc.dram_tensor("out", [NPAIR * P, 2048], F32, kind="ExternalOutput").ap()
    with tile.TileContext(nc) as tc:
        with ExitStack() as ctx:
            _emit(nc, tc, ctx, xl, wq, wk, wv, idr, out)
    if compile:
        nc.compile()
    _cache[key] = nc
    return nc


def kernel(x, w_query, w_key, w_value, _want_trace=False):
    x = np.ascontiguousarray(np.asarray(x, np.float32))
    wqa = np.ascontiguousarray(np.asarray(w_query, np.float32))
    wka = np.ascontiguousarray(np.asarray(w_key, np.float32))
    wva = np.ascontiguousarray(np.asarray(w_value, np.float32))
    B, S, E = x.shape
    xf = x.reshape(B * S, E)
    nc = build()
    rows = NPAIR * P
    wq_t = np.ascontiguousarray(
        (wqa * SCALE).reshape(NJ, P, NJ, P).transpose(2, 1, 0, 3)
    )
    wk_t = np.ascontiguousarray(wka.reshape(NJ, P, NJ, P).transpose(2, 1, 0, 3))
    wv_t = np.ascontiguousarray(
        wva.reshape(NJ, P, 4, 512).transpose(2, 1, 0, 3).astype(ml_dtypes.bfloat16)
    )
    eye = np.eye(P, dtype=np.float32)
    in_maps = [
        dict(xl=np.ascontiguousarray(xf[c * rows:(c + 1) * rows]),
             wq=wq_t, wk=wk_t, wv=wv_t, idr=eye)
        for c in range(8)
    ]
    res = run_bass_kernel_spmd(nc, in_maps, core_ids=list(range(8)),
                               trace=_want_trace)
    outf = np.concatenate([r["out"] for r in res.results], axis=0)
    if _want_trace:
        kernel.last_result = res
    return outf.reshape(B, S, E)


# revision 21
# speedup vs baseline: 1.4238x; 1.2093x over previous
"""MultiHeadAttention Trainium2 Bass kernel, 8-core SPMD. v3.

Problem: B=4, S=2048, E=2048, H=16, Dh=128; reshape-based (not transposed)
head split:  q = (x@Wq).reshape(B,H,S,Dh) etc., softmax over the QUERY axis,
out = attn.reshape(B,S,E).

Key structure: flattening (B,S) rows, row-block gp (128 rows) of x@W is
exactly head pair gp=(b,h): Qh = Y[128gp:128gp+128,:].reshape(2048,128).
Each of the 8 cores handles 8 consecutive pairs -> core c gets contiguous
x rows [1024c:1024c+1024) and produces the same output rows. No collectives.

Per-core internal q/k index permutation (order-free since softmax reduces
over q): f = j*128 + s  <->  q = 16s + j.

v3 design:
- ONE group of 8 pairs: Wq/Wk/Wv each streamed once.
- X-transpose and V-projection interleaved per pair (Wv bf16, resident).
- 1/sqrt(128) scale folded into Wq on the host.
- Q^T/K^T spilled to DRAM f32r (d-major), reloaded per pair.
- PSUM: psSC 3x[128,1024] (scores, 1.5-step lookahead) + psAT 2x[128,512]
  (attention accumulated in quarter-tiles, drains overlap next quarter's
  matmuls) = 8 banks.
- Attention two pairs staggered; sum/reciprocal/V-scale deferred one step
  (step_post) so ACT-dependent ops don't block next step's maxes in the
  in-order DVE queue; burst quarters spread over steps 9..12, drains
  deferred and alternated DVE/ACT.
- DMA queues: weights/x/qt/kt/out on sync, spills on gpsimd.
"""

import numpy as np
import ml_dtypes
from contextlib import ExitStack

import concourse.bass as bass
import concourse.tile as tile
from concourse import bacc, mybir
from concourse.bass import ds, ts
from concourse.bass_utils import run_bass_kernel_spmd
from concourse.masks import make_identity

F32 = mybir.dt.float32
F32R = mybir.dt.float32r
BF16 = mybir.dt.bfloat16
AX = mybir.AxisListType.X
EXP = mybir.ActivationFunctionType.Exp
MIN = mybir.AluOpType.min

P = 128
NPAIR = 8          # (b,h) pairs per core
NJ = 16            # 128-blocks in E / contraction
SCALE = 1.0 / np.sqrt(128.0)

_cache = {}


class _PairState:
    __slots__ = ("gp", "qt", "kt", "yv", "acc", "softs", "vss", "lsums", "pas")

    def __init__(self, gp, qt, kt, yv, acc):
        self.gp, self.qt, self.kt, self.yv, self.acc = gp, qt, kt, yv, acc
        self.softs, self.vss, self.lsums = {}, {}, {}
        self.pas = {}


def _emit(nc, tc, ctx, xl, wq, wk, wv, idr, out):
    sb = ctx.enter_context
    dram = sb(tc.tile_pool(name="dram", bufs=1, space="DRAM"))
    # d-major spill layout: [d, pair, j, s]
    qsp = dram.tile([P, NPAIR, NJ, P], F32R, tag="qsp")
    ksp = dram.tile([P, NPAIR, NJ, P], F32R, tag="ksp")

    # PSUM: one pool, 4x[128,1024] = 8 banks. Scores take 2 tiles/step
    # (depth-2 lookahead); attention quarters/finish borrow a tile briefly.
    psSC = sb(tc.tile_pool(name="pssc", bufs=4, space="PSUM"))

    pCONST = sb(tc.tile_pool(name="pconst", bufs=1))
    ident = pCONST.tile([P, P], F32, tag="ident")
    make_identity(nc, ident[:])
    identr = pCONST.tile([P, P], F32R, tag="identr")
    nc.sync.dma_start(identr[:], idr)

    pYV = sb(tc.tile_pool(name="pyv", bufs=NPAIR))
    yv_tiles = [
        pYV.tile([P, NJ * P], BF16, tag="yv", name=f"yv{i}") for i in range(NPAIR)
    ]

    with tc.tile_pool(name="pxtg", bufs=1) as pXTG:
        xtg = pXTG.tile([P, NJ, NPAIR, P], F32R, tag="xtg")
        # ---- phase XV: per pair, transpose x block then project V ----
        with tc.tile_pool(name="pxin", bufs=2) as pIN, \
             tc.tile_pool(name="pxt16", bufs=2) as pXT16, \
             tc.tile_pool(name="pwv", bufs=4) as pWV:
            wvts = []
            for c in range(4):
                wvt = pWV.tile([P, NJ, 512], BF16, tag="wv", name=f"wv{c}")
                nc.sync.dma_start(wvt[:], wv[c])
                wvts.append(wvt)
            for pi in range(NPAIR):
                xt = pIN.tile([P, NJ * P], F32R, tag="xt")
                nc.sync.dma_start(xt[:], xl[ds(pi * P, P), :])
                # xt16: [in-dim-block kb, s] transposed copy of this pair, bf16
                xt16 = pXT16.tile([P, NJ, P], BF16, tag="xt16")
                for jj in range(2):
                    pt = psSC.tile([P, 1024], F32, tag="sc")
                    for i in range(8):
                        j = jj * 8 + i
                        nc.tensor.transpose(
                            pt[:, ds(i * P, P)].bitcast(F32R),
                            xt[:, ds(j * P, P)], identr[:],
                        )
                    nc.vector.tensor_copy(
                        xtg[:, ts(jj, 8), pi, :],
                        pt[:].bitcast(F32R).rearrange("p (a b) -> p a b", a=8),
                    )
                    nc.scalar.copy(
                        xt16[:, ts(jj, 8), :],
                        pt[:].rearrange("p (a b) -> p a b", a=8),
                    )
                for c in range(4):
                    ps = psSC.tile([P, 1024], F32, tag="sc")
                    for kb in range(NJ):
                        nc.tensor.matmul(
                            ps[:, ds(0, 512)], xt16[:, kb], wvts[c][:, kb],
                            start=(kb == 0), stop=(kb == NJ - 1),
                        )
                    nc.scalar.copy(yv_tiles[pi][:, ds(c * 512, 512)], ps[:, ds(0, 512)])

        # ---- phase QK: Q^T/K^T projections, spill to DRAM ----
        with tc.tile_pool(name="pw", bufs=3) as pW, \
             tc.tile_pool(name="pstg", bufs=4) as pSTG:
            ci = 0
            for wd, sp in ((wq, qsp), (wk, ksp)):
                for j in range(NJ):
                    wt = pW.tile([P, NJ, P], F32R, tag="wqk")
                    nc.sync.dma_start(wt[:], wd[j])
                    ps = psSC.tile([P, 1024], F32, tag="sc")
                    for h in range(2):
                        for kb in range(NJ):
                            nc.tensor.matmul(
                                ps[:, ds(h * 512, 512)], wt[:, kb],
                                xtg[:, kb, ds(h * 4, 4), :],
                                start=(kb == 0), stop=(kb == NJ - 1),
                            )
                    stg = pSTG.tile([P, NPAIR, P], F32R, tag="stg")
                    if ci % 2 == 0:
                        nc.vector.tensor_copy(
                            stg[:], ps[:].rearrange("p (g s) -> p g s", g=NPAIR)
                        )
                    else:
                        nc.scalar.copy(
                            stg[:], ps[:].rearrange("p (g s) -> p g s", g=NPAIR)
                        )
                    ci += 1
                    nc.gpsimd.dma_start(sp[:, :, j, :], stg[:])

    # ---- attention, two pairs staggered ----
    with tc.tile_pool(name="pqt", bufs=3) as pQT, \
         tc.tile_pool(name="pkt", bufs=3) as pKT, \
         tc.tile_pool(name="psoft", bufs=40) as pSOFT, \
         tc.tile_pool(name="pvs", bufs=26) as pVS, \
         tc.tile_pool(name="pacc", bufs=2) as pACC, \
         tc.tile_pool(name="pst", bufs=36) as pST:

        def load(gp):
            qt = pQT.tile([P, NJ, P], F32R, tag="qt", name=f"qt{gp}")
            nc.sync.dma_start(qt[:], qsp[:, gp])
            kt = pKT.tile([P, NJ, P], F32R, tag="kt", name=f"kt{gp}")
            nc.sync.dma_start(kt[:], ksp[:, gp])
            return qt, kt

        def make_state(gp, qtkt):
            acc = pACC.tile([P, NJ * P], F32, tag="acc", name=f"acc{gp}")
            return _PairState(gp, qtkt[0], qtkt[1], yv_tiles[gp], acc)

        def step_pre(st, kj):
            """Scores matmuls, column maxes, combine, exps (+accum sums).
            soft stored as two [128,1024] half-tiles so attention quarters
            release h0 halves early (avoids pool-cycle deadlock)."""
            softs = []
            pss = []
            for h in range(2):
                ps = psSC.tile([P, 1024], F32, tag="sc")
                for c in range(2):
                    nc.tensor.matmul(
                        ps[:, ds(c * 512, 512)], st.kt[:, kj],
                        st.qt[:, ts(h * 2 + c, 4)], start=True, stop=True,
                    )
                pss.append(ps)
            ng = pST.tile([P, 1], F32, tag="ng")
            nm0 = pST.tile([P, 1], F32, tag="m0")
            nc.vector.reduce_max(nm0[:], pss[0][:], axis=AX, negate=True)
            nm1 = pST.tile([P, 1], F32, tag="nm1")
            nc.vector.reduce_max(nm1[:], pss[1][:], axis=AX, negate=True)
            nc.vector.tensor_tensor(ng[:], nm0[:], nm1[:], MIN)
            lsum = pST.tile([P, 2], F32, tag="ls")
            for h in range(2):
                soft = pSOFT.tile([P, 1024], BF16, tag="soft")
                nc.scalar.activation(
                    soft[:], pss[h][:], EXP,
                    bias=ng[:], scale=1.0, accum_out=lsum[:, ds(h, 1)],
                )
                softs.append(soft)
            st.softs[kj] = softs
            st.lsums[kj] = lsum

        def step_post(st, kj):
            """Deferred sum/reciprocal/V-scale (one step later so these
            ACT-dependent ops don't stall the in-order DVE queue)."""
            lsum = st.lsums.pop(kj)
            lt = pST.tile([P, 1], F32, tag="lt")
            nc.vector.reduce_sum(lt[:], lsum[:], axis=AX)
            rcp = pST.tile([P, 1], F32, tag="rcp")
            nc.vector.reciprocal(rcp[:], lt[:])
            vs = pVS.tile([P, P], BF16, tag="vs")
            nc.vector.tensor_scalar_mul(vs[:], st.yv[:, ts(kj, P)], rcp[:])
            st.vss[kj] = vs

        qno = [0]

        def quarter(st, g0, h, c):
            """One [128,512] attention-accumulation quarter over 8 kj."""
            pa = psSC.tile([P, 1024], F32, tag="sc")
            for i in range(8):
                kj = g0 + i
                nc.tensor.matmul(
                    pa[:, ds(0, 512)], st.vss[kj][:],
                    st.softs[kj][h][:, ds(c * 512, 512)],
                    start=(i == 0), stop=(i == 7),
                )
            st.pas[(g0, h, c)] = pa

        def drain(st, g0, h, c):
            pa = st.pas.pop((g0, h, c))
            dst = st.acc[:, ds(h * 1024 + c * 512, 512)]
            if g0 == 0:
                if qno[0] % 2 == 0:
                    nc.scalar.copy(dst, pa[:, ds(0, 512)])
                else:
                    nc.vector.tensor_copy(dst, pa[:, ds(0, 512)])
            else:
                nc.vector.tensor_add(dst, dst, pa[:, ds(0, 512)])
            qno[0] += 1

        QSCHED = {9: (0, 0), 10: (0, 1), 11: (1, 0), 12: (1, 1)}

        def finish(st):
            acc = st.acc
            for jj in range(2):
                pt = psSC.tile([P, 1024], F32, tag="sc")
                for i in range(8):
                    cblk = jj * 8 + i
                    nc.tensor.transpose(
                        pt[:, ds(i * P, P)], acc[:, ds(cblk * P, P)], ident[:]
                    )
                if jj % 2 == 0:
                    nc.scalar.copy(acc[:, ds(jj * 1024, 1024)], pt[:])
                else:
                    nc.vector.tensor_copy(acc[:, ds(jj * 1024, 1024)], pt[:])
            nc.sync.dma_start(out[ds(st.gp * P, P), :], acc[:])

        qtkts = {}
        qtkts[0], qtkts[1] = load(0), load(1)
        for d in range(4):
            pa_, pb_ = 2 * d, 2 * d + 1
            if d < 3:
                qtkts[pa_ + 2] = load(pa_ + 2)
            stA = make_state(pa_, qtkts.pop(pa_))
            stB = make_state(pb_, qtkts.pop(pb_))
            for kj in range(NJ):
                if kj - 1 in QSCHED:
                    drain(stA, 0, *QSCHED[kj - 1])
                if kj in QSCHED:
                    quarter(stA, 0, *QSCHED[kj])
                step_pre(stA, kj)
                if kj > 0:
                    step_post(stA, kj - 1)
                if kj - 1 in QSCHED:
                    drain(stB, 0, *QSCHED[kj - 1])
                if kj in QSCHED:
                    quarter(stB, 0, *QSCHED[kj])
                step_pre(stB, kj)
                if kj > 0:
                    step_post(stB, kj - 1)
                if d < 3 and kj == 8:
                    qtkts[pb_ + 2] = load(pb_ + 2)
            step_post(stA, NJ - 1)
            step_post(stB, NJ - 1)
            # tail: group-1 quarters, interleaved A/B, drains one behind
            tail = [(0, 0), (0, 1), (1, 0), (1, 1)]
            for qi, (h, c) in enumerate(tail):
                if qi > 0:
                    drain(stA, 8, *tail[qi - 1])
                quarter(stA, 8, h, c)
                if qi > 0:
                    drain(stB, 8, *tail[qi - 1])
                quarter(stB, 8, h, c)
            drain(stA, 8, 1, 1)
            finish(stA)
            drain(stB, 8, 1, 1)
            finish(stB)


def build(compile=True):
    key = ("nc_v3", compile)
    if key in _cache:
        return _cache[key]
    nc = bacc.Bacc("TRN2", target_bir_lowering=False, debug=False)
    xl = nc.dram_tensor("xl", [NPAIR * P, 2048], F32R, kind="ExternalInput").ap()
    wq = nc.dram_tensor("wq", [NJ, P, NJ, P], F32R, kind="ExternalInput").ap()
    wk = nc.dram_tensor("wk", [NJ, P, NJ, P], F32R, kind="ExternalInput").ap()
    wv = nc.dram_tensor("wv", [4, P, NJ, 512], BF16, kind="ExternalInput").ap()
    idr = nc.dram_tensor("idr", [P, P], F32R, kind="ExternalInput").ap()
    out = nc.dram_tensor("out", [NPAIR * P, 2048], F32, kind="ExternalOutput").ap()
    with tile.TileContext(nc) as tc:
        with ExitStack() as ctx:
            _emit(nc, tc, ctx, xl, wq, wk, wv, idr, out)
    if compile:
        nc.compile()
    _cache[key] = nc
    return nc


def kernel(x, w_query, w_key, w_value, _want_trace=False):
    x = np.ascontiguousarray(np.asarray(x, np.float32))
    wqa = np.ascontiguousarray(np.asarray(w_query, np.float32))
    wka = np.ascontiguousarray(np.asarray(w_key, np.float32))
    wva = np.ascontiguousarray(np.asarray(w_value, np.float32))
    B, S, E = x.shape
    xf = x.reshape(B * S, E)
    nc = build()
    rows = NPAIR * P
    wq_t = np.ascontiguousarray(
        (wqa * SCALE).reshape(NJ, P, NJ, P).transpose(2, 1, 0, 3)
    )
    wk_t = np.ascontiguousarray(wka.reshape(NJ, P, NJ, P).transpose(2, 1, 0, 3))
    wv_t = np.ascontiguousarray(
        wva.reshape(NJ, P, 4, 512).transpose(2, 1, 0, 3).astype(ml_dtypes.bfloat16)
    )
    eye = np.eye(P, dtype=np.float32)
    in_maps = [
        dict(xl=np.ascontiguousarray(xf[c * rows:(c + 1) * rows]),
             wq=wq_t, wk=wk_t, wv=wv_t, idr=eye)
        for c in range(8)
    ]
    res = run_bass_kernel_spmd(nc, in_maps, core_ids=list(range(8)),
                               trace=_want_trace)
    outf = np.concatenate([r["out"] for r in res.results], axis=0)
    if _want_trace:
        kernel.last_result = res
    return outf.reshape(B, S, E)


# revision 23
# speedup vs baseline: 1.4248x; 1.0007x over previous
"""MultiHeadAttention Trainium2 Bass kernel, 8-core SPMD. v4.

Problem: B=4, S=2048, E=2048, H=16, Dh=128; reshape-based (not transposed)
head split:  q = (x@Wq).reshape(B,H,S,Dh) etc., softmax over the QUERY axis,
out = attn.reshape(B,S,E).

Key structure: flattening (B,S) rows, row-block gp (128 rows) of x@W is
exactly head pair gp=(b,h): Qh = Y[128gp:128gp+128,:].reshape(2048,128).
Each of the 8 cores handles 8 consecutive pairs -> core c gets contiguous
x rows [1024c:1024c+1024) and produces the same output rows. No collectives.

Per-core internal q/k index permutation (order-free since softmax reduces
over q): f = j*128 + s  <->  q = 16s + j.

v4 design (1192us baseline -> ~941us measured):
- ONE group of 8 pairs: Wq/Wk/Wv each streamed once (48MB -> 40MB with
  Wv in bf16; V path is linear so bf16 is safe for the 2e-2 gate).
- X-transpose and V-projection interleaved per pair (Wv resident in SBUF).
- 1/sqrt(128) scale folded into Wq on the host.
- Q^T/K^T spilled to DRAM f32r (d-major layout), reloaded per pair.
- PSUM: ONE pool of 4x[128,1024] = 8 banks shared by scores (2 tiles per
  step -> depth-2 lookahead against the ~5us mm->max->min->exp chain),
  attention quarter-accumulators, and output transposes.
- Attention two pairs staggered; soft stored as per-half [128,1024] bf16
  tiles so the (h0,*) quarters at kj 9-10 release buffers early (this
  plus drain/quarter-before-step ordering avoids pool-cycle deadlocks);
  sum/reciprocal/V-scale deferred one step (step_post) so ACT-dependent
  ops don't block the next step's maxes in the in-order DVE queue.
- Copies balanced DVE/ACT; DMA issue split: weights/x/qt/kt/out on the
  sync queue, spills on the gpsimd queue (keeps any one sequencer from
  serializing on semaphore waits).
"""

import numpy as np
import ml_dtypes
from contextlib import ExitStack

import concourse.bass as bass
import concourse.tile as tile
from concourse import bacc, mybir
from concourse.bass import ds, ts
from concourse.bass_utils import run_bass_kernel_spmd
from concourse.masks import make_identity

F32 = mybir.dt.float32
F32R = mybir.dt.float32r
BF16 = mybir.dt.bfloat16
AX = mybir.AxisListType.X
EXP = mybir.ActivationFunctionType.Exp
MIN = mybir.AluOpType.min

P = 128
NPAIR = 8          # (b,h) pairs per core
NJ = 16            # 128-blocks in E / contraction
SCALE = 1.0 / np.sqrt(128.0)

_cache = {}


class _PairState:
    __slots__ = ("gp", "qt", "kt", "yv", "acc", "softs", "vss", "lsums", "pas")

    def __init__(self, gp, qt, kt, yv, acc):
        self.gp, self.qt, self.kt, self.yv, self.acc = gp, qt, kt, yv, acc
        self.softs, self.vss, self.lsums = {}, {}, {}
        self.pas = {}


def _emit(nc, tc, ctx, xl, wq, wk, wv, idr, out):
    sb = ctx.enter_context
    dram = sb(tc.tile_pool(name="dram", bufs=1, space="DRAM"))
    # d-major spill layout: [d, pair, j, s]
    qsp = dram.tile([P, NPAIR, NJ, P], F32R, tag="qsp")
    ksp = dram.tile([P, NPAIR, NJ, P], F32R, tag="ksp")

    # PSUM: one pool, 4x[128,1024] = 8 banks. Scores take 2 tiles/step
    # (depth-2 lookahead); attention quarters/finish borrow a tile briefly.
    psSC = sb(tc.tile_pool(name="pssc", bufs=4, space="PSUM"))

    pCONST = sb(tc.tile_pool(name="pconst", bufs=1))
    ident = pCONST.tile([P, P], F32, tag="ident")
    make_identity(nc, ident[:])
    identr = pCONST.tile([P, P], F32R, tag="identr")
    nc.sync.dma_start(identr[:], idr)

    pYV = sb(tc.tile_pool(name="pyv", bufs=NPAIR))
    yv_tiles = [
        pYV.tile([P, NJ * P], BF16, tag="yv", name=f"yv{i}") for i in range(NPAIR)
    ]

    with tc.tile_pool(name="pxtg", bufs=1) as pXTG:
        xtg = pXTG.tile([P, NJ, NPAIR, P], F32R, tag="xtg")
        # ---- phase XV: per pair, transpose x block then project V ----
        with tc.tile_pool(name="pxin", bufs=2) as pIN, \
             tc.tile_pool(name="pxt16", bufs=2) as pXT16, \
             tc.tile_pool(name="pwv", bufs=4) as pWV:
            wvts = []
            for c in range(4):
                wvt = pWV.tile([P, NJ, 512], BF16, tag="wv", name=f"wv{c}")
                nc.sync.dma_start(wvt[:], wv[c])
                wvts.append(wvt)
            for pi in range(NPAIR):
                xt = pIN.tile([P, NJ * P], F32R, tag="xt")
                nc.sync.dma_start(xt[:], xl[ds(pi * P, P), :])
                # xt16: [in-dim-block kb, s] transposed copy of this pair, bf16
                xt16 = pXT16.tile([P, NJ, P], BF16, tag="xt16")
                for jj in range(2):
                    pt = psSC.tile([P, 1024], F32, tag="sc")
                    for i in range(8):
                        j = jj * 8 + i
                        nc.tensor.transpose(
                            pt[:, ds(i * P, P)].bitcast(F32R),
                            xt[:, ds(j * P, P)], identr[:],
                        )
                    nc.vector.tensor_copy(
                        xtg[:, ts(jj, 8), pi, :],
                        pt[:].bitcast(F32R).rearrange("p (a b) -> p a b", a=8),
                    )
                    nc.scalar.copy(
                        xt16[:, ts(jj, 8), :],
                        pt[:].rearrange("p (a b) -> p a b", a=8),
                    )
                for c in range(4):
                    ps = psSC.tile([P, 1024], F32, tag="sc")
                    for kb in range(NJ):
                        nc.tensor.matmul(
                            ps[:, ds(0, 512)], xt16[:, kb], wvts[c][:, kb],
                            start=(kb == 0), stop=(kb == NJ - 1),
                        )
                    nc.scalar.copy(yv_tiles[pi][:, ds(c * 512, 512)], ps[:, ds(0, 512)])

        # ---- phase QK: Q^T/K^T projections, spill to DRAM ----
        with tc.tile_pool(name="pw", bufs=3) as pW, \
             tc.tile_pool(name="pstg", bufs=4) as pSTG:
            ci = 0
            for wd, sp in ((wq, qsp), (wk, ksp)):
                for j in range(NJ):
                    wt = pW.tile([P, NJ, P], F32R, tag="wqk")
                    nc.sync.dma_start(wt[:], wd[j])
                    ps = psSC.tile([P, 1024], F32, tag="sc")
                    for h in range(2):
                        for kb in range(NJ):
                            nc.tensor.matmul(
                                ps[:, ds(h * 512, 512)], wt[:, kb],
                                xtg[:, kb, ds(h * 4, 4), :],
                                start=(kb == 0), stop=(kb == NJ - 1),
                            )
                    stg = pSTG.tile([P, NPAIR, P], F32R, tag="stg")
                    if ci % 2 == 0:
                        nc.vector.tensor_copy(
                            stg[:], ps[:].rearrange("p (g s) -> p g s", g=NPAIR)
                        )
                    else:
                        nc.scalar.copy(
                            stg[:], ps[:].rearrange("p (g s) -> p g s", g=NPAIR)
                        )
                    ci += 1
                    nc.gpsimd.dma_start(sp[:, :, j, :], stg[:])

    # ---- attention, two pairs staggered ----
    with tc.tile_pool(name="pqt", bufs=3) as pQT, \
         tc.tile_pool(name="pkt", bufs=3) as pKT, \
         tc.tile_pool(name="psoft", bufs=40) as pSOFT, \
         tc.tile_pool(name="pvs", bufs=26) as pVS, \
         tc.tile_pool(name="pacc", bufs=2) as pACC, \
         tc.tile_pool(name="pst", bufs=36) as pST:

        def load(gp):
            qt = pQT.tile([P, NJ, P], F32R, tag="qt", name=f"qt{gp}")
            nc.sync.dma_start(qt[:], qsp[:, gp])
            kt = pKT.tile([P, NJ, P], F32R, tag="kt", name=f"kt{gp}")
            nc.sync.dma_start(kt[:], ksp[:, gp])
            return qt, kt

        def make_state(gp, qtkt):
            acc = pACC.tile([P, NJ * P], F32, tag="acc", name=f"acc{gp}")
            return _PairState(gp, qtkt[0], qtkt[1], yv_tiles[gp], acc)

        def step_pre(st, kj):
            """Scores matmuls, column maxes, combine, exps (+accum sums).
            soft stored as two [128,1024] half-tiles so attention quarters
            release h0 halves early (avoids pool-cycle deadlock)."""
            softs = []
            pss = []
            for h in range(2):
                ps = psSC.tile([P, 1024], F32, tag="sc")
                for c in range(2):
                    nc.tensor.matmul(
                        ps[:, ds(c * 512, 512)], st.kt[:, kj],
                        st.qt[:, ts(h * 2 + c, 4)], start=True, stop=True,
                    )
                pss.append(ps)
            ng = pST.tile([P, 1], F32, tag="ng")
            nm0 = pST.tile([P, 1], F32, tag="m0")
            nc.vector.reduce_max(nm0[:], pss[0][:], axis=AX, negate=True)
            nm1 = pST.tile([P, 1], F32, tag="nm1")
            nc.vector.reduce_max(nm1[:], pss[1][:], axis=AX, negate=True)
            nc.vector.tensor_tensor(ng[:], nm0[:], nm1[:], MIN)
            lsum = pST.tile([P, 2], F32, tag="ls")
            for h in range(2):
                soft = pSOFT.tile([P, 1024], BF16, tag="soft")
                nc.scalar.activation(
                    soft[:], pss[h][:], EXP,
                    bias=ng[:], scale=1.0, accum_out=lsum[:, ds(h, 1)],
                )
                softs.append(soft)
            st.softs[kj] = softs
            st.lsums[kj] = lsum

        def step_post(st, kj):
            """Deferred sum/reciprocal/V-scale (one step later so these
            ACT-dependent ops don't stall the in-order DVE queue)."""
            lsum = st.lsums.pop(kj)
            lt = pST.tile([P, 1], F32, tag="lt")
            nc.vector.reduce_sum(lt[:], lsum[:], axis=AX)
            rcp = pST.tile([P, 1], F32, tag="rcp")
            nc.vector.reciprocal(rcp[:], lt[:])
            vs = pVS.tile([P, P], BF16, tag="vs")
            nc.vector.tensor_scalar_mul(vs[:], st.yv[:, ts(kj, P)], rcp[:])
            st.vss[kj] = vs

        qno = [0]

        def quarter(st, g0, h, c):
            """One [128,512] attention-accumulation quarter over 8 kj."""
            pa = psSC.tile([P, 1024], F32, tag="sc")
            for i in range(8):
                kj = g0 + i
                nc.tensor.matmul(
                    pa[:, ds(0, 512)], st.vss[kj][:],
                    st.softs[kj][h][:, ds(c * 512, 512)],
                    start=(i == 0), stop=(i == 7),
                )
            st.pas[(g0, h, c)] = pa

        def drain(st, g0, h, c):
            pa = st.pas.pop((g0, h, c))
            dst = st.acc[:, ds(h * 1024 + c * 512, 512)]
            if g0 == 0:
                if qno[0] % 2 == 0:
                    nc.scalar.copy(dst, pa[:, ds(0, 512)])
                else:
                    nc.vector.tensor_copy(dst, pa[:, ds(0, 512)])
            else:
                nc.vector.tensor_add(dst, dst, pa[:, ds(0, 512)])
            qno[0] += 1

        QSCHED = {9: (0, 0), 10: (0, 1), 11: (1, 0), 12: (1, 1)}

        def finish(st):
            acc = st.acc
            for jj in range(2):
                pt = psSC.tile([P, 1024], F32, tag="sc")
                for i in range(8):
                    cblk = jj * 8 + i
                    nc.tensor.transpose(
                        pt[:, ds(i * P, P)], acc[:, ds(cblk * P, P)], ident[:]
                    )
                if jj % 2 == 0:
                    nc.scalar.copy(acc[:, ds(jj * 1024, 1024)], pt[:])
                else:
                    nc.vector.tensor_copy(acc[:, ds(jj * 1024, 1024)], pt[:])
            nc.sync.dma_start(out[ds(st.gp * P, P), :], acc[:])

        qtkts = {}
        qtkts[0], qtkts[1] = load(0), load(1)
        for d in range(4):
            pa_, pb_ = 2 * d, 2 * d + 1
            if d < 3:
                qtkts[pa_ + 2] = load(pa_ + 2)
            stA = make_state(pa_, qtkts.pop(pa_))
            stB = make_state(pb_, qtkts.pop(pb_))
            for kj in range(NJ):
                if kj - 1 in QSCHED:
                    drain(stA, 0, *QSCHED[kj - 1])
                if kj in QSCHED:
                    quarter(stA, 0, *QSCHED[kj])
                step_pre(stA, kj)
                if kj > 0:
                    step_post(stA, kj - 1)
                if kj - 1 in QSCHED:
                    drain(stB, 0, *QSCHED[kj - 1])
                if kj in QSCHED:
                    quarter(stB, 0, *QSCHED[kj])
                step_pre(stB, kj)
                if kj > 0:
                    step_post(stB, kj - 1)
                if d < 3 and kj == 8:
                    qtkts[pb_ + 2] = load(pb_ + 2)
            step_post(stA, NJ - 1)
            step_post(stB, NJ - 1)
            # tail: group-1 quarters, interleaved A/B, drains one behind
            tail = [(0, 0), (0, 1), (1, 0), (1, 1)]
            for qi, (h, c) in enumerate(tail):
                if qi > 0:
                    drain(stA, 8, *tail[qi - 1])
                quarter(stA, 8, h, c)
                if qi > 0:
                    drain(stB, 8, *tail[qi - 1])
                quarter(stB, 8, h, c)
            drain(stA, 8, 1, 1)
            finish(stA)
            drain(stB, 8, 1, 1)
            finish(stB)


def build(compile=True):
    key = ("nc_v3", compile)
    if key in _cache:
        return _cache[key]
    nc = bacc.Bacc("TRN2", target_bir_lowering=False, debug=False)
    xl = nc.dram_tensor("xl", [NPAIR * P, 2048], F32R, kind="ExternalInput").ap()
    wq = nc.dram_tensor("wq", [NJ, P, NJ, P], F32R, kind="ExternalInput").ap()
    wk = nc.dram_tensor("wk", [NJ, P, NJ, P], F32R, kind="ExternalInput").ap()
    wv = nc.dram_tensor("wv", [4, P, NJ, 512], BF16, kind="ExternalInput").ap()
    idr = nc.dram_tensor("idr", [P, P], F32R, kind="ExternalInput").ap()
    out = nc.dram_tensor("out", [NPAIR * P, 2048], F32, kind="ExternalOutput").ap()
    with tile.TileContext(nc) as tc:
        with ExitStack() as ctx:
            _emit(nc, tc, ctx, xl, wq, wk, wv, idr, out)
    if compile:
        nc.compile()
    _cache[key] = nc
    return nc


def kernel(x, w_query, w_key, w_value, _want_trace=False):
    x = np.ascontiguousarray(np.asarray(x, np.float32))
    wqa = np.ascontiguousarray(np.asarray(w_query, np.float32))
    wka = np.ascontiguousarray(np.asarray(w_key, np.float32))
    wva = np.ascontiguousarray(np.asarray(w_value, np.float32))
    B, S, E = x.shape
    xf = x.reshape(B * S, E)
    nc = build()
    rows = NPAIR * P
    wq_t = np.ascontiguousarray(
        (wqa * SCALE).reshape(NJ, P, NJ, P).transpose(2, 1, 0, 3)
    )
    wk_t = np.ascontiguousarray(wka.reshape(NJ, P, NJ, P).transpose(2, 1, 0, 3))
    wv_t = np.ascontiguousarray(
        wva.reshape(NJ, P, 4, 512).transpose(2, 1, 0, 3).astype(ml_dtypes.bfloat16)
    )
    eye = np.eye(P, dtype=np.float32)
    in_maps = [
        dict(xl=np.ascontiguousarray(xf[c * rows:(c + 1) * rows]),
             wq=wq_t, wk=wk_t, wv=wv_t, idr=eye)
        for c in range(8)
    ]
    res = run_bass_kernel_spmd(nc, in_maps, core_ids=list(range(8)),
                               trace=_want_trace)
    outf = np.concatenate([r["out"] for r in res.results], axis=0)
    if _want_trace:
        kernel.last_result = res
    return outf.reshape(B, S, E)


# revision 31
# speedup vs baseline: 1.4965x; 1.0503x over previous
"""MultiHeadAttention Trainium2 Bass kernel, 8-core SPMD. v4.

Problem: B=4, S=2048, E=2048, H=16, Dh=128; reshape-based (not transposed)
head split:  q = (x@Wq).reshape(B,H,S,Dh) etc., softmax over the QUERY axis,
out = attn.reshape(B,S,E).

Key structure: flattening (B,S) rows, row-block gp (128 rows) of x@W is
exactly head pair gp=(b,h): Qh = Y[128gp:128gp+128,:].reshape(2048,128).
Each of the 8 cores handles 8 consecutive pairs -> core c gets contiguous
x rows [1024c:1024c+1024) and produces the same output rows. No collectives.

Per-core internal q/k index permutation (order-free since softmax reduces
over q): f = j*128 + s  <->  q = 16s + j.

v4 design (1192us baseline -> ~941us measured):
- ONE group of 8 pairs: Wq/Wk/Wv each streamed once (48MB -> 40MB with
  Wv in bf16; V path is linear so bf16 is safe for the 2e-2 gate).
- X-transpose and V-projection interleaved per pair (Wv resident in SBUF).
- 1/sqrt(128) scale folded into Wq on the host.
- Q^T/K^T spilled to DRAM f32r (d-major layout), reloaded per pair.
- PSUM: ONE pool of 4x[128,1024] = 8 banks shared by scores (2 tiles per
  step -> depth-2 lookahead against the ~5us mm->max->min->exp chain),
  attention quarter-accumulators, and output transposes.
- Attention two pairs staggered; soft stored as per-half [128,1024] bf16
  tiles so the (h0,*) quarters at kj 9-10 release buffers early (this
  plus drain/quarter-before-step ordering avoids pool-cycle deadlocks);
  sum/reciprocal/V-scale deferred one step (step_post) so ACT-dependent
  ops don't block the next step's maxes in the in-order DVE queue.
- Copies balanced DVE/ACT; DMA issue split: weights/x/qt/kt/out on the
  sync queue, spills on the gpsimd queue (keeps any one sequencer from
  serializing on semaphore waits).
"""

import numpy as np
import ml_dtypes
from contextlib import ExitStack

import concourse.bass as bass
import concourse.tile as tile
from concourse import bacc, mybir
from concourse.bass import ds, ts
from concourse.bass_utils import run_bass_kernel_spmd
from concourse.masks import make_identity

F32 = mybir.dt.float32
F32R = mybir.dt.float32r
BF16 = mybir.dt.bfloat16
AX = mybir.AxisListType.X
EXP = mybir.ActivationFunctionType.Exp
MIN = mybir.AluOpType.min

P = 128
NPAIR = 8          # (b,h) pairs per core
NJ = 16            # 128-blocks in E / contraction
SCALE = 1.0 / np.sqrt(128.0)

_cache = {}


class _PairState:
    __slots__ = ("gp", "qt", "kt", "yv", "acc", "softs", "vss", "lsums", "pas")

    def __init__(self, gp, qt, kt, yv, acc):
        self.gp, self.qt, self.kt, self.yv, self.acc = gp, qt, kt, yv, acc
        self.softs, self.vss, self.lsums = {}, {}, {}
        self.pas = {}


def _emit(nc, tc, ctx, xl, wq, wk, wv, idr, out):
    sb = ctx.enter_context
    dram = sb(tc.tile_pool(name="dram", bufs=1, space="DRAM"))
    # d-major spill layout: [d, pair, j, s]
    qsp = dram.tile([P, NPAIR, NJ, P], F32R, tag="qsp")
    ksp = dram.tile([P, NPAIR, NJ, P], F32R, tag="ksp")

    # PSUM: one pool, 4x[128,1024] = 8 banks. Scores take 2 tiles/step
    # (depth-2 lookahead); attention quarters/finish borrow a tile briefly.
    psSC = sb(tc.tile_pool(name="pssc", bufs=4, space="PSUM"))

    pCONST = sb(tc.tile_pool(name="pconst", bufs=1))
    ident = pCONST.tile([P, P], F32, tag="ident")
    make_identity(nc, ident[:])
    identr = pCONST.tile([P, P], F32R, tag="identr")
    nc.sync.dma_start(identr[:], idr)

    pYV = sb(tc.tile_pool(name="pyv", bufs=NPAIR))
    yv_tiles = [
        pYV.tile([P, NJ * P], BF16, tag="yv", name=f"yv{i}") for i in range(NPAIR)
    ]

    with tc.tile_pool(name="pxtg", bufs=1) as pXTG:
        xtg = pXTG.tile([P, NJ, NPAIR, P], F32R, tag="xtg")
        # ---- phase XV: per pair, transpose x block then project V ----
        with tc.tile_pool(name="pxin", bufs=2) as pIN, \
             tc.tile_pool(name="pxt16", bufs=2) as pXT16, \
             tc.tile_pool(name="pwv", bufs=4) as pWV:
            wvts = []
            for c in range(4):
                wvt = pWV.tile([P, NJ, 512], BF16, tag="wv", name=f"wv{c}")
                nc.sync.dma_start(wvt[:], wv[c])
                wvts.append(wvt)
            for pi in range(NPAIR):
                xt = pIN.tile([P, NJ * P], F32R, tag="xt")
                nc.sync.dma_start(xt[:], xl[ds(pi * P, P), :])
                # xt16: [in-dim-block kb, s] transposed copy of this pair, bf16
                xt16 = pXT16.tile([P, NJ, P], BF16, tag="xt16")
                for jj in range(2):
                    pt = psSC.tile([P, 1024], F32, tag="sc")
                    for i in range(8):
                        j = jj * 8 + i
                        nc.tensor.transpose(
                            pt[:, ds(i * P, P)].bitcast(F32R),
                            xt[:, ds(j * P, P)], identr[:],
                        )
                    nc.vector.tensor_copy(
                        xtg[:, ts(jj, 8), pi, :],
                        pt[:].bitcast(F32R).rearrange("p (a b) -> p a b", a=8),
                    )
                    nc.scalar.copy(
                        xt16[:, ts(jj, 8), :],
                        pt[:].rearrange("p (a b) -> p a b", a=8),
                    )
                for c in range(4):
                    ps = psSC.tile([P, 1024], F32, tag="sc")
                    for kb in range(NJ):
                        nc.tensor.matmul(
                            ps[:, ds(0, 512)], xt16[:, kb], wvts[c][:, kb],
                            start=(kb == 0), stop=(kb == NJ - 1),
                        )
                    nc.scalar.copy(yv_tiles[pi][:, ds(c * 512, 512)], ps[:, ds(0, 512)])

        # ---- phase QK: Q^T/K^T projections, spill to DRAM ----
        with tc.tile_pool(name="pw", bufs=3) as pW, \
             tc.tile_pool(name="pstg", bufs=4) as pSTG:
            ci = 0
            for wd, sp in ((wq, qsp), (wk, ksp)):
                for j in range(NJ):
                    wt = pW.tile([P, NJ, P], F32R, tag="wqk")
                    nc.sync.dma_start(wt[:], wd[j])
                    ps = psSC.tile([P, 1024], F32, tag="sc")
                    for h in range(2):
                        for kb in range(NJ):
                            nc.tensor.matmul(
                                ps[:, ds(h * 512, 512)], wt[:, kb],
                                xtg[:, kb, ds(h * 4, 4), :],
                                start=(kb == 0), stop=(kb == NJ - 1),
                            )
                    stg = pSTG.tile([P, NPAIR, P], F32R, tag="stg")
                    if ci % 2 == 0:
                        nc.vector.tensor_copy(
                            stg[:], ps[:].rearrange("p (g s) -> p g s", g=NPAIR)
                        )
                    else:
                        nc.scalar.copy(
                            stg[:], ps[:].rearrange("p (g s) -> p g s", g=NPAIR)
                        )
                    ci += 1
                    nc.gpsimd.dma_start(sp[:, :, j, :], stg[:])

    # ---- attention, two pairs staggered ----
    with tc.tile_pool(name="pqt", bufs=3) as pQT, \
         tc.tile_pool(name="pkt", bufs=3) as pKT, \
         tc.tile_pool(name="psoft", bufs=40) as pSOFT, \
         tc.tile_pool(name="pvs", bufs=56) as pVS, \
         tc.tile_pool(name="pacc", bufs=2) as pACC, \
         tc.tile_pool(name="pst", bufs=40) as pST:

        def load(gp):
            qt = pQT.tile([P, NJ, P], F32R, tag="qt", name=f"qt{gp}")
            nc.sync.dma_start(qt[:], qsp[:, gp])
            kt = pKT.tile([P, NJ, P], F32R, tag="kt", name=f"kt{gp}")
            nc.sync.dma_start(kt[:], ksp[:, gp])
            return qt, kt

        def make_state(gp, qtkt):
            acc = pACC.tile([P, NJ * P], F32, tag="acc", name=f"acc{gp}")
            return _PairState(gp, qtkt[0], qtkt[1], yv_tiles[gp], acc)

        def step_pre(st, kj):
            """Scores matmuls, per-half column max, per-half exp (+accum).
            Each exp is biased by ITS OWN half's max, so it waits on only
            one reduce -- the halves are recombined exactly in post1/post2
            via per-half scale factors folded into the V-scale tiles."""
            softs = []
            pss = []
            for h in range(2):
                ps = psSC.tile([P, 1024], F32, tag="sc")
                for c in range(2):
                    nc.tensor.matmul(
                        ps[:, ds(c * 512, 512)], st.kt[:, kj],
                        st.qt[:, ts(h * 2 + c, 4)], start=True, stop=True,
                    )
                pss.append(ps)
            nms = []
            for h in range(2):
                nm = pST.tile([P, 1], F32, tag="nm")
                nc.vector.reduce_max(nm[:], pss[h][:], axis=AX, negate=True)
                nms.append(nm)
            lsum = pST.tile([P, 2], F32, tag="ls")
            for h in range(2):
                soft = pSOFT.tile([P, 1024], BF16, tag="soft")
                nc.scalar.activation(
                    soft[:], pss[h][:], EXP,
                    bias=nms[h][:], scale=1.0, accum_out=lsum[:, ds(h, 1)],
                )
                softs.append(soft)
            st.softs[kj] = softs
            st.lsums[kj] = (lsum, nms)

        def step_post1(st, kj):
            """Global max combine + per-half rescale factors e_h=exp(m_h-M)."""
            lsum, nms = st.lsums[kj]
            ng = pST.tile([P, 1], F32, tag="ng")
            nc.vector.tensor_tensor(ng[:], nms[0][:], nms[1][:], MIN)
            es = []
            for h in range(2):
                e = pST.tile([P, 1], F32, tag="e")
                nc.scalar.activation(e[:], nms[h][:], EXP, bias=ng[:], scale=-1.0)
                es.append(e)
            st.lsums[kj] = (lsum, es)

        def step_post2(st, kj):
            """L = l0*e0 + l1*e1, rcp, per-half V-scale tiles (GpSimd does
            the tiny scalar algebra so the DVE queue never waits on ACT)."""
            lsum, es = st.lsums.pop(kj)
            lw0 = pST.tile([P, 1], F32, tag="lw0")
            nc.gpsimd.tensor_scalar_mul(lw0[:], lsum[:, ds(0, 1)], es[0][:])
            lw1 = pST.tile([P, 1], F32, tag="lw1")
            nc.gpsimd.tensor_scalar_mul(lw1[:], lsum[:, ds(1, 1)], es[1][:])
            lt = pST.tile([P, 1], F32, tag="lt")
            nc.gpsimd.tensor_tensor(lt[:], lw0[:], lw1[:], mybir.AluOpType.add)
            rcp = pST.tile([P, 1], F32, tag="rcp")
            nc.vector.reciprocal(rcp[:], lt[:])
            vss = []
            for h in range(2):
                rr = pST.tile([P, 1], F32, tag="rr")
                nc.gpsimd.tensor_scalar_mul(rr[:], es[h][:], rcp[:])
                vs = pVS.tile([P, P], BF16, tag="vs")
                nc.vector.tensor_scalar_mul(vs[:], st.yv[:, ts(kj, P)], rr[:])
                vss.append(vs)
            st.vss[kj] = vss

        qno = [0]

        def quarter(st, g0, h, c):
            """One [128,512] attention-accumulation quarter over 8 kj."""
            pa = psSC.tile([P, 1024], F32, tag="sc")
            for i in range(8):
                kj = g0 + i
                nc.tensor.matmul(
                    pa[:, ds(0, 512)], st.vss[kj][h][:],
                    st.softs[kj][h][:, ds(c * 512, 512)],
                    start=(i == 0), stop=(i == 7),
                )
            st.pas[(g0, h, c)] = pa

        def drain(st, g0, h, c):
            pa = st.pas.pop((g0, h, c))
            dst = st.acc[:, ds(h * 1024 + c * 512, 512)]
            if g0 == 0:
                if qno[0] % 2 == 0:
                    nc.scalar.copy(dst, pa[:, ds(0, 512)])
                else:
                    nc.vector.tensor_copy(dst, pa[:, ds(0, 512)])
            else:
                nc.vector.tensor_add(dst, dst, pa[:, ds(0, 512)])
            qno[0] += 1

        QSCHED = {9: (0, 0), 10: (0, 1), 11: (1, 0), 12: (1, 1)}

        def finish(st):
            acc = st.acc
            for jj in range(2):
                pt = psSC.tile([P, 1024], F32, tag="sc")
                for i in range(8):
                    cblk = jj * 8 + i
                    nc.tensor.transpose(
                        pt[:, ds(i * P, P)], acc[:, ds(cblk * P, P)], ident[:]
                    )
                if jj % 2 == 0:
                    nc.scalar.copy(acc[:, ds(jj * 1024, 1024)], pt[:])
                else:
                    nc.vector.tensor_copy(acc[:, ds(jj * 1024, 1024)], pt[:])
            nc.sync.dma_start(out[ds(st.gp * P, P), :], acc[:])

        qtkts = {}
        qtkts[0], qtkts[1] = load(0), load(1)
        for d in range(4):
            pa_, pb_ = 2 * d, 2 * d + 1
            if d < 3:
                qtkts[pa_ + 2] = load(pa_ + 2)
            stA = make_state(pa_, qtkts.pop(pa_))
            stB = make_state(pb_, qtkts.pop(pb_))
            for kj in range(NJ):
                if kj > 0:
                    step_post1(stA, kj - 1)
                if kj > 1:
                    step_post2(stA, kj - 2)
                if kj - 1 in QSCHED:
                    drain(stA, 0, *QSCHED[kj - 1])
                if kj in QSCHED:
                    quarter(stA, 0, *QSCHED[kj])
                step_pre(stA, kj)
                if kj > 0:
                    step_post1(stB, kj - 1)
                if kj > 1:
                    step_post2(stB, kj - 2)
                if kj - 1 in QSCHED:
                    drain(stB, 0, *QSCHED[kj - 1])
                if kj in QSCHED:
                    quarter(stB, 0, *QSCHED[kj])
                step_pre(stB, kj)
                if d < 3 and kj == 8:
                    qtkts[pb_ + 2] = load(pb_ + 2)
            step_post1(stA, NJ - 1)
            step_post2(stA, NJ - 2)
            step_post2(stA, NJ - 1)
            step_post1(stB, NJ - 1)
            step_post2(stB, NJ - 2)
            step_post2(stB, NJ - 1)
            # tail: group-1 quarters, interleaved A/B, drains one behind
            tail = [(0, 0), (0, 1), (1, 0), (1, 1)]
            for qi, (h, c) in enumerate(tail):
                if qi > 0:
                    drain(stA, 8, *tail[qi - 1])
                quarter(stA, 8, h, c)
                if qi > 0:
                    drain(stB, 8, *tail[qi - 1])
                quarter(stB, 8, h, c)
            drain(stA, 8, 1, 1)
            finish(stA)
            drain(stB, 8, 1, 1)
            finish(stB)


def build(compile=True):
    key = ("nc_v3", compile)
    if key in _cache:
        return _cache[key]
    nc = bacc.Bacc("TRN2", target_bir_lowering=False, debug=False)
    xl = nc.dram_tensor("xl", [NPAIR * P, 2048], F32R, kind="ExternalInput").ap()
    wq = nc.dram_tensor("wq", [NJ, P, NJ, P], F32R, kind="ExternalInput").ap()
    wk = nc.dram_tensor("wk", [NJ, P, NJ, P], F32R, kind="ExternalInput").ap()
    wv = nc.dram_tensor("wv", [4, P, NJ, 512], BF16, kind="ExternalInput").ap()
    idr = nc.dram_tensor("idr", [P, P], F32R, kind="ExternalInput").ap()
    out = nc.dram_tensor("out", [NPAIR * P, 2048], F32, kind="ExternalOutput").ap()
    with tile.TileContext(nc) as tc:
        with ExitStack() as ctx:
            _emit(nc, tc, ctx, xl, wq, wk, wv, idr, out)
    if compile:
        nc.compile()
    _cache[key] = nc
    return nc


def kernel(x, w_query, w_key, w_value, _want_trace=False):
    x = np.ascontiguousarray(np.asarray(x, np.float32))
    wqa = np.ascontiguousarray(np.asarray(w_query, np.float32))
    wka = np.ascontiguousarray(np.asarray(w_key, np.float32))
    wva = np.ascontiguousarray(np.asarray(w_value, np.float32))
    B, S, E = x.shape
    xf = x.reshape(B * S, E)
    nc = build()
    rows = NPAIR * P
    wq_t = np.ascontiguousarray(
        (wqa * SCALE).reshape(NJ, P, NJ, P).transpose(2, 1, 0, 3)
    )
    wk_t = np.ascontiguousarray(wka.reshape(NJ, P, NJ, P).transpose(2, 1, 0, 3))
    wv_t = np.ascontiguousarray(
        wva.reshape(NJ, P, 4, 512).transpose(2, 1, 0, 3).astype(ml_dtypes.bfloat16)
    )
    eye = np.eye(P, dtype=np.float32)
    in_maps = [
        dict(xl=np.ascontiguousarray(xf[c * rows:(c + 1) * rows]),
             wq=wq_t, wk=wk_t, wv=wv_t, idr=eye)
        for c in range(8)
    ]
    res = run_bass_kernel_spmd(nc, in_maps, core_ids=list(range(8)),
                               trace=_want_trace)
    outf = np.concatenate([r["out"] for r in res.results], axis=0)
    if _want_trace:
        kernel.last_result = res
    return outf.reshape(B, S, E)


# revision 35
# speedup vs baseline: 1.4990x; 1.0017x over previous
"""MultiHeadAttention Trainium2 Bass kernel, 8-core SPMD. v5.

Problem: B=4, S=2048, E=2048, H=16, Dh=128; reshape-based (not transposed)
head split:  q = (x@Wq).reshape(B,H,S,Dh) etc., softmax over the QUERY axis,
out = attn.reshape(B,S,E).

Key structure: flattening (B,S) rows, row-block gp (128 rows) of x@W is
exactly head pair gp=(b,h): Qh = Y[128gp:128gp+128,:].reshape(2048,128).
Each of the 8 cores handles 8 consecutive pairs -> core c gets contiguous
x rows [1024c:1024c+1024) and produces the same output rows. No collectives.

Per-core internal q/k index permutation (order-free since softmax reduces
over q): f = j*128 + s  <->  q = 16s + j.

v5 design (1192us baseline -> ~896us measured):
- Decoupled softmax halves: each half's exp is biased by its OWN column
  max (one reduce on its critical path instead of two plus a combine);
  the halves are recombined EXACTLY via e_h = exp(m_h - M) folded into
  per-half V-scale tiles (vs_h = V * e_h / L), with the tiny scalar
  algebra on the otherwise-idle GpSimd queue and deferred two steps so
  the in-order DVE/ACT queues never stall on it.
- ONE group of 8 pairs: Wq/Wk/Wv each streamed once (48MB -> 40MB with
  Wv in bf16; V path is linear so bf16 is safe for the 2e-2 gate).
- X-transpose and V-projection interleaved per pair (Wv resident in SBUF).
- 1/sqrt(128) scale folded into Wq on the host.
- Q^T/K^T spilled to DRAM f32r (d-major layout), reloaded per pair.
- PSUM: ONE pool of 4x[128,1024] = 8 banks shared by scores (2 tiles per
  step -> depth-2 lookahead against the ~5us mm->max->min->exp chain),
  attention quarter-accumulators, and output transposes.
- Attention two pairs staggered; soft stored as per-half [128,1024] bf16
  tiles so the (h0,*) quarters at kj 9-10 release buffers early (this
  plus drain/quarter-before-step ordering avoids pool-cycle deadlocks);
  sum/reciprocal/V-scale deferred one step (step_post) so ACT-dependent
  ops don't block the next step's maxes in the in-order DVE queue.
- Copies balanced DVE/ACT; DMA issue split: weights/x/qt/kt/out on the
  sync queue, spills on the gpsimd queue (keeps any one sequencer from
  serializing on semaphore waits).
"""

import numpy as np
import ml_dtypes
from contextlib import ExitStack

import concourse.bass as bass
import concourse.tile as tile
from concourse import bacc, mybir
from concourse.bass import ds, ts
from concourse.bass_utils import run_bass_kernel_spmd
from concourse.masks import make_identity

F32 = mybir.dt.float32
F32R = mybir.dt.float32r
BF16 = mybir.dt.bfloat16
AX = mybir.AxisListType.X
EXP = mybir.ActivationFunctionType.Exp
MIN = mybir.AluOpType.min

P = 128
NPAIR = 8          # (b,h) pairs per core
NJ = 16            # 128-blocks in E / contraction
SCALE = 1.0 / np.sqrt(128.0)

_cache = {}


class _PairState:
    __slots__ = ("gp", "qt", "kt", "yv", "acc", "softs", "vss", "lsums", "pas")

    def __init__(self, gp, qt, kt, yv, acc):
        self.gp, self.qt, self.kt, self.yv, self.acc = gp, qt, kt, yv, acc
        self.softs, self.vss, self.lsums = {}, {}, {}
        self.pas = {}


def _emit(nc, tc, ctx, xl, wq, wk, wv, idr, out):
    sb = ctx.enter_context
    dram = sb(tc.tile_pool(name="dram", bufs=1, space="DRAM"))
    # d-major spill layout: [d, pair, j, s]
    qsp = dram.tile([P, NPAIR, NJ, P], F32R, tag="qsp")
    ksp = dram.tile([P, NPAIR, NJ, P], F32R, tag="ksp")

    # PSUM: one pool, 4x[128,1024] = 8 banks. Scores take 2 tiles/step
    # (depth-2 lookahead); attention quarters/finish borrow a tile briefly.
    psSC = sb(tc.tile_pool(name="pssc", bufs=4, space="PSUM"))

    pCONST = sb(tc.tile_pool(name="pconst", bufs=1))
    ident = pCONST.tile([P, P], F32, tag="ident")
    make_identity(nc, ident[:])
    identr = pCONST.tile([P, P], F32R, tag="identr")
    nc.sync.dma_start(identr[:], idr)

    pYV = sb(tc.tile_pool(name="pyv", bufs=NPAIR))
    yv_tiles = [
        pYV.tile([P, NJ * P], BF16, tag="yv", name=f"yv{i}") for i in range(NPAIR)
    ]

    with tc.tile_pool(name="pxtg", bufs=1) as pXTG:
        xtg = pXTG.tile([P, NJ, NPAIR, P], F32R, tag="xtg")
        # ---- phase XV: per pair, transpose x block then project V ----
        with tc.tile_pool(name="pxin", bufs=2) as pIN, \
             tc.tile_pool(name="pxt16", bufs=2) as pXT16, \
             tc.tile_pool(name="pwv", bufs=4) as pWV:
            wvts = []
            for c in range(4):
                wvt = pWV.tile([P, NJ, 512], BF16, tag="wv", name=f"wv{c}")
                nc.sync.dma_start(wvt[:], wv[c])
                wvts.append(wvt)
            for pi in range(NPAIR):
                xt = pIN.tile([P, NJ * P], F32R, tag="xt")
                nc.sync.dma_start(xt[:], xl[ds(pi * P, P), :])
                # xt16: [in-dim-block kb, s] transposed copy of this pair, bf16
                xt16 = pXT16.tile([P, NJ, P], BF16, tag="xt16")
                for jj in range(2):
                    pt = psSC.tile([P, 1024], F32, tag="sc")
                    for i in range(8):
                        j = jj * 8 + i
                        nc.tensor.transpose(
                            pt[:, ds(i * P, P)].bitcast(F32R),
                            xt[:, ds(j * P, P)], identr[:],
                        )
                    nc.vector.tensor_copy(
                        xtg[:, ts(jj, 8), pi, :],
                        pt[:].bitcast(F32R).rearrange("p (a b) -> p a b", a=8),
                    )
                    nc.scalar.copy(
                        xt16[:, ts(jj, 8), :],
                        pt[:].rearrange("p (a b) -> p a b", a=8),
                    )
                for c in range(4):
                    ps = psSC.tile([P, 1024], F32, tag="sc")
                    for kb in range(NJ):
                        nc.tensor.matmul(
                            ps[:, ds(0, 512)], xt16[:, kb], wvts[c][:, kb],
                            start=(kb == 0), stop=(kb == NJ - 1),
                        )
                    nc.scalar.copy(yv_tiles[pi][:, ds(c * 512, 512)], ps[:, ds(0, 512)])

        # ---- phase QK: Q^T/K^T projections, spill to DRAM ----
        with tc.tile_pool(name="pw", bufs=3) as pW, \
             tc.tile_pool(name="pstg", bufs=4) as pSTG:
            ci = 0
            for wd, sp in ((wq, qsp), (wk, ksp)):
                for j in range(NJ):
                    wt = pW.tile([P, NJ, P], F32R, tag="wqk")
                    nc.sync.dma_start(wt[:], wd[j])
                    ps = psSC.tile([P, 1024], F32, tag="sc")
                    for h in range(2):
                        for kb in range(NJ):
                            nc.tensor.matmul(
                                ps[:, ds(h * 512, 512)], wt[:, kb],
                                xtg[:, kb, ds(h * 4, 4), :],
                                start=(kb == 0), stop=(kb == NJ - 1),
                            )
                    stg = pSTG.tile([P, NPAIR, P], F32R, tag="stg")
                    if ci % 2 == 0:
                        nc.vector.tensor_copy(
                            stg[:], ps[:].rearrange("p (g s) -> p g s", g=NPAIR)
                        )
                    else:
                        nc.scalar.copy(
                            stg[:], ps[:].rearrange("p (g s) -> p g s", g=NPAIR)
                        )
                    ci += 1
                    nc.gpsimd.dma_start(sp[:, :, j, :], stg[:])

    # ---- attention, two pairs staggered ----
    with tc.tile_pool(name="pqt", bufs=3) as pQT, \
         tc.tile_pool(name="pkt", bufs=3) as pKT, \
         tc.tile_pool(name="psoft", bufs=40) as pSOFT, \
         tc.tile_pool(name="pvs", bufs=60) as pVS, \
         tc.tile_pool(name="pacc", bufs=2) as pACC, \
         tc.tile_pool(name="pst", bufs=40) as pST:

        def load(gp):
            qt = pQT.tile([P, NJ, P], F32R, tag="qt", name=f"qt{gp}")
            nc.sync.dma_start(qt[:], qsp[:, gp])
            kt = pKT.tile([P, NJ, P], F32R, tag="kt", name=f"kt{gp}")
            nc.sync.dma_start(kt[:], ksp[:, gp])
            return qt, kt

        def make_state(gp, qtkt):
            acc = pACC.tile([P, NJ * P], F32, tag="acc", name=f"acc{gp}")
            return _PairState(gp, qtkt[0], qtkt[1], yv_tiles[gp], acc)

        def step_pre(st, kj):
            """Scores matmuls, per-half column max, per-half exp (+accum).
            Each exp is biased by ITS OWN half's max, so it waits on only
            one reduce -- the halves are recombined exactly in post1/post2
            via per-half scale factors folded into the V-scale tiles."""
            softs = []
            pss = []
            for h in range(2):
                ps = psSC.tile([P, 1024], F32, tag="sc")
                for c in range(2):
                    nc.tensor.matmul(
                        ps[:, ds(c * 512, 512)], st.kt[:, kj],
                        st.qt[:, ts(h * 2 + c, 4)], start=True, stop=True,
                    )
                pss.append(ps)
            nms = []
            for h in range(2):
                nm = pST.tile([P, 1], F32, tag="nm")
                nc.vector.reduce_max(nm[:], pss[h][:], axis=AX, negate=True)
                nms.append(nm)
            lsum = pST.tile([P, 2], F32, tag="ls")
            for h in range(2):
                soft = pSOFT.tile([P, 1024], BF16, tag="soft")
                nc.scalar.activation(
                    soft[:], pss[h][:], EXP,
                    bias=nms[h][:], scale=1.0, accum_out=lsum[:, ds(h, 1)],
                )
                softs.append(soft)
            st.softs[kj] = softs
            st.lsums[kj] = (lsum, nms)

        def step_post1(st, kj):
            """Global max combine + per-half rescale factors e_h=exp(m_h-M)."""
            lsum, nms = st.lsums[kj]
            ng = pST.tile([P, 1], F32, tag="ng")
            nc.vector.tensor_tensor(ng[:], nms[0][:], nms[1][:], MIN)
            es = []
            for h in range(2):
                e = pST.tile([P, 1], F32, tag="e")
                nc.scalar.activation(e[:], nms[h][:], EXP, bias=ng[:], scale=-1.0)
                es.append(e)
            st.lsums[kj] = (lsum, es)

        def step_post2(st, kj):
            """L = l0*e0 + l1*e1, rcp, per-half V-scale tiles (GpSimd does
            the tiny scalar algebra so the DVE queue never waits on ACT)."""
            lsum, es = st.lsums.pop(kj)
            lw0 = pST.tile([P, 1], F32, tag="lw0")
            nc.gpsimd.tensor_scalar_mul(lw0[:], lsum[:, ds(0, 1)], es[0][:])
            lw1 = pST.tile([P, 1], F32, tag="lw1")
            nc.gpsimd.tensor_scalar_mul(lw1[:], lsum[:, ds(1, 1)], es[1][:])
            lt = pST.tile([P, 1], F32, tag="lt")
            nc.gpsimd.tensor_tensor(lt[:], lw0[:], lw1[:], mybir.AluOpType.add)
            rcp = pST.tile([P, 1], F32, tag="rcp")
            nc.vector.reciprocal(rcp[:], lt[:])
            vss = []
            for h in range(2):
                rr = pST.tile([P, 1], F32, tag="rr")
                nc.gpsimd.tensor_scalar_mul(rr[:], es[h][:], rcp[:])
                vs = pVS.tile([P, P], BF16, tag="vs")
                nc.vector.tensor_scalar_mul(vs[:], st.yv[:, ts(kj, P)], rr[:])
                vss.append(vs)
            st.vss[kj] = vss

        qno = [0]

        def quarter(st, g0, h, c):
            """One [128,512] attention-accumulation quarter over 8 kj."""
            pa = psSC.tile([P, 1024], F32, tag="sc")
            for i in range(8):
                kj = g0 + i
                nc.tensor.matmul(
                    pa[:, ds(0, 512)], st.vss[kj][h][:],
                    st.softs[kj][h][:, ds(c * 512, 512)],
                    start=(i == 0), stop=(i == 7),
                )
            st.pas[(g0, h, c)] = pa

        def drain(st, g0, h, c):
            pa = st.pas.pop((g0, h, c))
            dst = st.acc[:, ds(h * 1024 + c * 512, 512)]
            if g0 == 0:
                if qno[0] % 2 == 0:
                    nc.scalar.copy(dst, pa[:, ds(0, 512)])
                else:
                    nc.vector.tensor_copy(dst, pa[:, ds(0, 512)])
            else:
                nc.vector.tensor_add(dst, dst, pa[:, ds(0, 512)])
            qno[0] += 1

        QSCHED = {9: (0, 0), 10: (0, 1), 12: (1, 0), 14: (1, 1)}

        def finish(st):
            acc = st.acc
            for jj in range(2):
                pt = psSC.tile([P, 1024], F32, tag="sc")
                for i in range(8):
                    cblk = jj * 8 + i
                    nc.tensor.transpose(
                        pt[:, ds(i * P, P)], acc[:, ds(cblk * P, P)], ident[:]
                    )
                if jj % 2 == 0:
                    nc.scalar.copy(acc[:, ds(jj * 1024, 1024)], pt[:])
                else:
                    nc.vector.tensor_copy(acc[:, ds(jj * 1024, 1024)], pt[:])
            nc.sync.dma_start(out[ds(st.gp * P, P), :], acc[:])

        qtkts = {}
        qtkts[0], qtkts[1] = load(0), load(1)
        for d in range(4):
            pa_, pb_ = 2 * d, 2 * d + 1
            if d < 3:
                qtkts[pa_ + 2] = load(pa_ + 2)
            stA = make_state(pa_, qtkts.pop(pa_))
            stB = make_state(pb_, qtkts.pop(pb_))
            for kj in range(NJ):
                if kj > 0:
                    step_post1(stA, kj - 1)
                if kj > 1:
                    step_post2(stA, kj - 2)
                if kj - 1 in QSCHED:
                    drain(stA, 0, *QSCHED[kj - 1])
                if kj in QSCHED:
                    quarter(stA, 0, *QSCHED[kj])
                step_pre(stA, kj)
                if kj > 0:
                    step_post1(stB, kj - 1)
                if kj > 1:
                    step_post2(stB, kj - 2)
                if kj - 1 in QSCHED:
                    drain(stB, 0, *QSCHED[kj - 1])
                if kj in QSCHED:
                    quarter(stB, 0, *QSCHED[kj])
                step_pre(stB, kj)
                if d < 3 and kj == 8:
                    qtkts[pb_ + 2] = load(pb_ + 2)
            step_post1(stA, NJ - 1)
            step_post2(stA, NJ - 2)
            step_post2(stA, NJ - 1)
            step_post1(stB, NJ - 1)
            step_post2(stB, NJ - 2)
            step_post2(stB, NJ - 1)
            # tail: group-1 quarters, interleaved A/B, drains one behind
            tail = [(0, 0), (0, 1), (1, 0), (1, 1)]
            for qi, (h, c) in enumerate(tail):
                if qi > 0:
                    drain(stA, 8, *tail[qi - 1])
                quarter(stA, 8, h, c)
                if qi > 0:
                    drain(stB, 8, *tail[qi - 1])
                quarter(stB, 8, h, c)
            drain(stA, 8, 1, 1)
            finish(stA)
            drain(stB, 8, 1, 1)
            finish(stB)


def build(compile=True):
    key = ("nc_v3", compile)
    if key in _cache:
        return _cache[key]
    nc = bacc.Bacc("TRN2", target_bir_lowering=False, debug=False)
    xl = nc.dram_tensor("xl", [NPAIR * P, 2048], F32R, kind="ExternalInput").ap()
    wq = nc.dram_tensor("wq", [NJ, P, NJ, P], F32R, kind="ExternalInput").ap()
    wk = nc.dram_tensor("wk", [NJ, P, NJ, P], F32R, kind="ExternalInput").ap()
    wv = nc.dram_tensor("wv", [4, P, NJ, 512], BF16, kind="ExternalInput").ap()
    idr = nc.dram_tensor("idr", [P, P], F32R, kind="ExternalInput").ap()
    out = nc.dram_tensor("out", [NPAIR * P, 2048], F32, kind="ExternalOutput").ap()
    with tile.TileContext(nc) as tc:
        with ExitStack() as ctx:
            _emit(nc, tc, ctx, xl, wq, wk, wv, idr, out)
    if compile:
        nc.compile()
    _cache[key] = nc
    return nc


def kernel(x, w_query, w_key, w_value, _want_trace=False):
    x = np.ascontiguousarray(np.asarray(x, np.float32))
    wqa = np.ascontiguousarray(np.asarray(w_query, np.float32))
    wka = np.ascontiguousarray(np.asarray(w_key, np.float32))
    wva = np.ascontiguousarray(np.asarray(w_value, np.float32))
    B, S, E = x.shape
    xf = x.reshape(B * S, E)
    nc = build()
    rows = NPAIR * P
    wq_t = np.ascontiguousarray(
        (wqa * SCALE).reshape(NJ, P, NJ, P).transpose(2, 1, 0, 3)
    )
    wk_t = np.ascontiguousarray(wka.reshape(NJ, P, NJ, P).transpose(2, 1, 0, 3))
    wv_t = np.ascontiguousarray(
        wva.reshape(NJ, P, 4, 512).transpose(2, 1, 0, 3).astype(ml_dtypes.bfloat16)
    )
    eye = np.eye(P, dtype=np.float32)
    in_maps = [
        dict(xl=np.ascontiguousarray(xf[c * rows:(c + 1) * rows]),
             wq=wq_t, wk=wk_t, wv=wv_t, idr=eye)
        for c in range(8)
    ]
    res = run_bass_kernel_spmd(nc, in_maps, core_ids=list(range(8)),
                               trace=_want_trace)
    outf = np.concatenate([r["out"] for r in res.results], axis=0)
    if _want_trace:
        kernel.last_result = res
    return outf.reshape(B, S, E)
